# revision 38
# baseline (speedup 1.0000x reference)
"""Trainium2 Bass kernel for nn_AttentionModule (B=4, C=256, 64x64 spatial).

Reference computation (per batch b, x flattened to [C, HW]):
    q = Wq @ x + bq            [32, HW]
    k = Wk @ x + bk            [32, HW]
    v = x^T @ Wv^T + bv        [HW, 256]
    out = softmax(q^T @ k) @ v [HW, 256] -> transposed to [C, HW]

Sharding: 8 cores, data-parallel over (batch, query-half): core = 2*b + h
computes queries [h*2048, (h+1)*2048) of batch b against all 4096 keys.
Weights replicated.

Numerics: fp16 inputs/projections, fp32 PSUM accumulate, bf16 attention
probabilities (scores reach +-40, exp in fp32 -> bf16, no max-subtraction).

Device layout (v3 — fast prologue + arrival-aware schedule; ~112us vs
v2's ~116.7us in the fast clock state):
  - scores transposed ([keys, q]) so the PE accumulates the softmax
    denominator itself: v carries a ones column, out[:, 256] = sum_k exp.
  - QK is 4-way row-packed: kT4 holds k^T in four 32-partition bands;
    qrep holds q^T replicated at all four bands. Each attention step
    runs 4 adjacent K=32 matmuls (tile_position rows 0/32/64/96) into
    one [128, 2048] psum tile — adjacency matters: K=32 row-banded
    matmuls co-execute in disjoint PE quadrants (~400ns for the quad)
    only when no full-array matmul sits between them and their deps
    are met together. One [128, 2048] EXP per step on ScalarE.
  - steady-state period = quad 400 + sem + EXP 1966 + sem ~= 2580ns
    (single score buffer; a ping-pong split was tried and measured
    equal: it breaks quad co-execution, +240ns/step, while saving the
    same amount of serialization).
  - band layouts are built IN PSUM via tile_position column offsets
    (stationary tile at array cols 32j writes psum partitions
    32j..32j+32), then one DVE copy to SBUF — no SBUF-to-SBUF DMA
    chain on the prologue critical path.
  - split step 0: kproj covers k chunks {0,1} first so QK(0) bands 0,1
    + their EXP half run on x cols 0:1024 while cols 1024:2048 land.
  - AV: P-stationary [q, 258] psum tiles, lag ONE step behind exp;
    A-sweep covers q-subtiles 0,1 inline; B-sweeps for chunks 0/1 are
    emitted as four 2-key-group slices at steps s0/s0+2/s0+4/s0+6
    (s0 = 9/17) so the later slices fill the chain-gapped steps of
    each window; chunk 2 is one burst at s=24 (its pp tiles must be
    normed before opsB[3] allocates at s=25 — finer slicing there
    deadlocks the in-order fetch against the 4-deep wait queue);
    chunk 3 runs all 4 subtiles inline. An 8-way 1-group-per-step
    dribble measured WORSE (+7us) than 2-group slices.
    Normalization = per-partition reciprocal + tensor_scalar multiply
    on VectorE (tail half on ScalarE).
  - input DMA: per-queue transfers serialize at ~75GB/s, so the
    QK(0)-gating 1MB (x cols 0:2048) is spread evenly over the three
    trigger queues (sync/scalar HWDGE + gpsimd SWDGE), first x pieces
    before the tiny wq|wk piece, wv before the back x pieces;
    projection work in the step loop is ordered by DMA arrival.
    NOTE: gpsimd SWDGE x triggers must not lead the program — an early
    SWDGE-first layout correlated with the chip entering a 1.2x-slower
    clock state for the whole run.
  - 8 warm-up matmuls release the HAM clock gate during the DMA window.
  - final [q, c] -> [c, q] transpose + bv bias happen host-side.
  - NOTE: exec time flips between a fast (~112us) and slow (~133us,
    uniform 1.2x on every engine) chip clock state across identical
    invocations; compare only fast-state minima when benchmarking.
"""
import numpy as np
from contextlib import ExitStack

import concourse.bass as bass
import concourse.bacc as bacc
import concourse.tile as tile
from concourse import mybir
from concourse.bass_utils import run_bass_kernel_spmd

B, C, H, W = 4, 256, 64, 64
HW = H * W            # 4096
D = C // 8            # 32 (q/k channels)
NCORES = 8
Q = HW // 2           # 2048 queries per core
P = 128
VW = C + 2            # v tile width (ones col + even-pad)
PC = 512              # projection chunk width

F32 = mybir.dt.float32
F16 = mybir.dt.float16
BF16 = mybir.dt.bfloat16
EXP = mybir.ActivationFunctionType.Exp

_CACHE: dict = {}


def build_program(with_bias: bool = False) -> bacc.Bacc:
    nc = bacc.Bacc("TRN2", target_bir_lowering=False, debug=False)

    # xkv is rolled per-core so the own query half occupies cols [0, 2048):
    # softmax(q k^T) v is invariant to key order, so kT4/v use rolled order too.
    xkv_d = nc.dram_tensor("xkv", [C, HW], F16, kind="ExternalInput").ap()
    # packed weights per c'-half: [wqT | wkT | wvT]
    WB = 2 * D + C        # 320
    wpk_d = nc.dram_tensor("wpk", [C, WB], F16, kind="ExternalInput").ap()
    # packed [bq | bk | ones(PC)]
    bpk_d = nc.dram_tensor("bpk", [1, 2 * D + PC], F16, kind="ExternalInput").ap()
    o_d = nc.dram_tensor("o", [Q, C], F16, kind="ExternalOutput").ap()

    with tile.TileContext(nc) as tc:
        with ExitStack() as ctx:
            big = ctx.enter_context(tc.tile_pool(name="big", bufs=14))
            const = ctx.enter_context(tc.tile_pool(name="const", bufs=1))
            ep = ctx.enter_context(tc.tile_pool(name="ep", bufs=4))
            ps = ctx.enter_context(tc.tile_pool(name="ps", bufs=1, space="PSUM"))
            pav = ctx.enter_context(tc.tile_pool(name="pav", bufs=2, space="PSUM"))
            pp = ctx.enter_context(tc.tile_pool(name="pp", bufs=2, space="PSUM"))

            # ---- PE warm-up: zeroed tile matmuls release the HAM clock
            # gate while the first input DMAs land ----
            dummy = const.tile([P, PC], F16, tag="dummy")
            nc.vector.memset(dummy[:], 0.0)
            for _ in range(8):
                wps = pp.tile([P, PC], F32, tag="pp", name="wps")
                nc.tensor.matmul(wps[:], dummy[:, 0:P], dummy[:],
                                 start=True, stop=True)

            # ---- input DMAs, consumption order, spread over the three
            # trigger queues (sync=SP, scalar=ACT HWDGE; gpsimd SWDGE).
            # Per-queue transfers serialize at ~75GB/s, so the
            # prologue-critical pieces go first on each queue. ----
            wpk_t = [const.tile([P, WB], F16, tag=f"wpk{i}", name=f"wpk{i}")
                     for i in range(2)]
            xkv_t = [const.tile([P, HW], F16, tag=f"xkv{i}", name=f"xkv{i}")
                     for i in range(2)]
            bpk_t = const.tile([1, 2 * D + PC], F16, tag="bpk")

            # The QK(0)-gating input (x cols 0:2048 of both halves, 1MB)
            # is spread evenly over the three trigger queues; per-queue
            # transfers serialize at ~75GB/s. First x pieces precede the
            # tiny wq|wk piece; wv sits before the back x pieces (the
            # first vprojs gate the AV pipeline start).
            eng = [nc.sync, nc.scalar]
            for i in range(2):
                eng[i].dma_start(xkv_t[i][:, 0:512], xkv_d[i * P:(i + 1) * P, 0:512])
            for i in range(2):
                eng[i].dma_start(wpk_t[i][:, 0:2 * D],
                                 wpk_d[i * P:(i + 1) * P, 0:2 * D])
            if with_bias:
                nc.sync.dma_start(bpk_t[:], bpk_d)
            for i in range(2):
                nc.gpsimd.dma_start(xkv_t[i][:, 1536:2048],
                                    xkv_d[i * P:(i + 1) * P, 1536:2048])
            for i in range(2):
                eng[i].dma_start(xkv_t[i][:, 512:1024],
                                 xkv_d[i * P:(i + 1) * P, 512:1024])
            for i in range(2):
                eng[1 - i].dma_start(xkv_t[i][:, 1024:1536],
                                     xkv_d[i * P:(i + 1) * P, 1024:1536])
            for i in range(2):
                eng[i].dma_start(wpk_t[i][:, 2 * D:WB],
                                 wpk_d[i * P:(i + 1) * P, 2 * D:WB])
            for i in range(2):
                eng[i].dma_start(xkv_t[i][:, 2048:3072],
                                 xkv_d[i * P:(i + 1) * P, 2048:3072])
            for i in range(2):
                eng[1 - i].dma_start(xkv_t[i][:, 3072:4096],
                                     xkv_d[i * P:(i + 1) * P, 3072:4096])

            wq_sb = [wpk_t[i][:, 0:D] for i in range(2)]
            wk_sb = [wpk_t[i][:, D:2 * D] for i in range(2)]
            wv_sb = [wpk_t[i][:, 2 * D:WB] for i in range(2)]
            bq_sb = bpk_t[:, 0:D]
            bk_sb = bpk_t[:, D:2 * D]
            ones_sb = bpk_t[:, 2 * D:]

            # kT4: band r (partitions 32r..32r+32) holds k chunks {r, 4+r};
            # QK step (ci, g) with g=(m,t) uses key tiles kt = 16m + 4r + t
            # on band r.
            kT4 = const.tile([P, 2 * PC], F16, tag="kT4")
            # qrep: q^T replicated at all four bands.
            qrep = const.tile([P, Q], F16, tag="qrep")
            v_all = const.tile([P, (HW // P) * VW], F16, tag="vall")
            nc.vector.memset(
                v_all[:].rearrange("p (k c) -> p k c", c=VW)[:, :, C:C + 2], 1.0)
            v_sb = [v_all[:, t * VW:(t + 1) * VW] for t in range(HW // P)]

            # ---- projections (PE, fp16 in / f32 psum) ----
            # Band-packed psum groups: pre-zeroed psum + all-accumulate
            # matmuls with tile_position col offsets (correct under any
            # execution order; the sim's one-group-per-2KB-region check
            # is bypassed).
            def kproj4(m, jlist=None):
                # k chunks 4m..4m+3 -> kT4[:, 512m:512m+512], band layout.
                kp = pp.tile([P, PC], F32, tag="pp", name="kp")
                nc.vector.memset(kp[:], 0.0)
                js = jlist if jlist is not None else range(4 * m, 4 * m + 4)
                mms = []
                for j in js:
                    for h in range(2):
                        mms.append((kp[32 * (j % 4):32 * (j % 4) + 32, :],
                                    wk_sb[h], xkv_t[h][:, PC * j:PC * (j + 1)],
                                    32 * (j % 4)))
                    if with_bias:
                        mms.append((kp[32 * (j % 4):32 * (j % 4) + 32, :],
                                    bk_sb, ones_sb, 32 * (j % 4)))
                for i, (out, lhs, rhs, cp) in enumerate(mms):
                    nc.tensor.matmul(out, lhs, rhs, start=False,
                                     stop=(i == len(mms) - 1),
                                     skip_group_check=True,
                                     tile_position=(0, cp))
                nc.vector.tensor_copy(kT4[:, PC * m:PC * (m + 1)], kp[:])

            def qproj4(j):
                # q chunk j computed 4x via col groups -> psum already
                # band-replicated; one DVE copy, no DMA.
                qp = pp.tile([P, PC], F32, tag="pp", name="qp")
                nc.vector.memset(qp[:], 0.0)
                nmm = 12 if with_bias else 8
                i = 0
                for h in range(2):
                    for r in range(4):
                        nc.tensor.matmul(
                            qp[32 * r:32 * (r + 1), :], wq_sb[h],
                            xkv_t[h][:, PC * j:PC * (j + 1)],
                            start=False, stop=(i == nmm - 1),
                            skip_group_check=True, tile_position=(0, 32 * r))
                        i += 1
                if with_bias:
                    for r in range(4):
                        nc.tensor.matmul(
                            qp[32 * r:32 * (r + 1), :], bq_sb, ones_sb,
                            start=False, stop=(i == nmm - 1),
                            skip_group_check=True, tile_position=(0, 32 * r))
                        i += 1
                nc.vector.tensor_copy(qrep[:, PC * j:PC * (j + 1)], qp[:])

            def qsolo(j):
                # q chunk j (throughput path): 2 matmuls into band 0, then
                # SWDGE replication to bands 1-3 (latency-tolerant).
                qp = pp.tile([D, PC], F32, tag="pp", name="qs")
                nc.tensor.matmul(qp[:], wq_sb[0], xkv_t[0][:, PC * j:PC * (j + 1)],
                                 start=True, stop=False)
                nc.tensor.matmul(qp[:], wq_sb[1], xkv_t[1][:, PC * j:PC * (j + 1)],
                                 start=False, stop=not with_bias)
                if with_bias:
                    nc.tensor.matmul(qp[:], bq_sb, ones_sb, start=False, stop=True)
                nc.vector.tensor_copy(qrep[0:D, PC * j:PC * (j + 1)], qp[:])
                for r in range(1, 4):
                    nc.gpsimd.dma_start(qrep[32 * r:32 * r + 32, PC * j:PC * (j + 1)],
                                        qrep[0:D, PC * j:PC * (j + 1)])

            def vproj(t):
                j, off = divmod(t, PC // P)
                vp = pp.tile([P, C], F32, tag="pp", name="vp")
                for xh in range(2):
                    nc.tensor.matmul(
                        vp[:], xkv_t[xh][:, PC * j + off * P:PC * j + (off + 1) * P],
                        wv_sb[xh], start=(xh == 0), stop=(xh == 1))
                nc.vector.tensor_copy(v_sb[t][:, 0:C], vp[:])

            def kproj2(h2):
                # k chunks {2*h2, 2*h2+1} -> kT4 rows 64*h2:64*h2+64,
                # block 0. Splitting kproj4(0) lets QK(0) bands 0,1 run
                # on x cols 0:1024 while cols 1024:2048 are still landing.
                kp = pp.tile([P, PC], F32, tag="pp", name="kp2")
                rows = slice(64 * h2, 64 * h2 + 64)
                nc.vector.memset(kp[rows, :], 0.0)
                mms = []
                # h2=1: chunk 3 (cols 1536:2048, gpsimd queue) lands
                # before chunk 2 (cols 1024:1536)
                for j in ((3, 2) if h2 == 1 else (0, 1)):
                    for h in range(2):
                        mms.append((kp[32 * j:32 * j + 32, :], wk_sb[h],
                                    xkv_t[h][:, PC * j:PC * (j + 1)], 32 * j))
                    if with_bias:
                        mms.append((kp[32 * j:32 * j + 32, :], bk_sb, ones_sb,
                                    32 * j))
                for i, (out, lhs, rhs, cp) in enumerate(mms):
                    nc.tensor.matmul(out, lhs, rhs, start=False,
                                     stop=(i == len(mms) - 1),
                                     skip_group_check=True,
                                     tile_position=(0, cp))
                nc.vector.tensor_copy(kT4[rows, 0:PC], kp[rows, :])

            # v tiles in AV consumption order (kt = 16m + 4r + t).
            vorder = [16 * m + 4 * r + t
                      for m in range(2) for t in range(4) for r in range(4)]
            # proj_work order tracks x DMA arrival: V4+ read x cols
            # 0:2048 (landed by the loop start); kproj4(1) reads cols
            # 2048:4096 which land ~12us, so it drains at s=2 (used by
            # QK step 4) — emitting it earlier stalls the in-order PE.
            V = [lambda t=t: vproj(t) for t in vorder]
            proj_work = ([lambda: qsolo(1)] + V[4:8]
                         + [lambda: qsolo(2)] + V[8:12]
                         + [lambda: qsolo(3)] + V[12:16]
                         + [lambda: kproj4(1)] + V[16:32])

            # ---- attention: 32 QK steps, AV lags by 1 ----
            def av_norm(op, ci, qb, last=False, act=False):
                rinv = ep.tile([P, 1], F32, tag="rinv", name="rinv")
                nc.vector.reciprocal(rinv[:], op[:, C:C + 1])
                osb = ep.tile([P, C], F16, tag="osb", name="osb")
                if act:
                    # ScalarE is idle after the final exp; runs in parallel
                    # with the DVE half of the last chunk's epilogue.
                    nc.scalar.mul(osb[:], op[:, 0:C], rinv[:])
                else:
                    nc.vector.tensor_scalar_mul(osb[:], op[:, 0:C], rinv[:])
                q0 = ci * 512 + qb * P
                e = nc.scalar if last else nc.sync
                e.dma_start(o_d[q0:q0 + P, :], osb[:])

            pair_tiles = {}
            opsA = {}
            opsB = {}
            wi = 0
            NST32 = 32
            opsBd = {}

            def qk_quad(s):
                ci, g = divmod(s, 8)
                m, t = divmod(g, 4)
                sc = ps.tile([P, 2048], F32, tag="p", name="sc")
                for r in range(4):
                    nc.tensor.matmul(
                        sc[:, 512 * r:512 * (r + 1)],
                        kT4[32 * r:32 * r + 32, PC * m + P * t:PC * m + P * (t + 1)],
                        qrep[32 * r:32 * r + 32, PC * ci:PC * (ci + 1)],
                        start=True, stop=True, tile_position=(32 * r, 0))
                return sc

            def av_half(s2, rlist, nq):
                ci2, g2 = divmod(s2, 8)
                m2, t2 = divmod(g2, 4)
                Pt2 = pair_tiles[s2]
                for r in rlist:
                    kt = 16 * m2 + 4 * r + t2
                    for qs in range(nq):
                        op = opsA[ci2][qs] if qs < 2 else opsB[3][qs - 2]
                        nc.tensor.matmul(
                            op[:],
                            Pt2[:, 512 * r + P * qs:512 * r + P * (qs + 1)],
                            v_sb[kt][:],
                            start=False, skip_group_check=True,
                            stop=(g2 == 7 and r == 3))

            # ---- split step 0: bands 0,1 (k chunks 0,1; x cols 0:1024)
            # run while cols 1024:2048 land; V0-V3 fill the wait. ----
            qproj4(0)
            kproj2(0)
            sc0 = ps.tile([P, 2048], F32, tag="p", name="sc")
            Pt0 = big.tile([P, 2048], BF16, tag="big", name="pt")
            for r in range(2):
                nc.tensor.matmul(
                    sc0[:, 512 * r:512 * (r + 1)],
                    kT4[32 * r:32 * r + 32, 0:P],
                    qrep[32 * r:32 * r + 32, 0:PC],
                    start=True, stop=True, tile_position=(32 * r, 0))
            nc.scalar.activation(Pt0[:, 0:1024], sc0[:, 0:1024], EXP)
            kproj2(1)
            vproj(vorder[0])
            vproj(vorder[1])
            for r in range(2, 4):
                nc.tensor.matmul(
                    sc0[:, 512 * r:512 * (r + 1)],
                    kT4[32 * r:32 * r + 32, 0:P],
                    qrep[32 * r:32 * r + 32, 0:PC],
                    start=True, stop=True, tile_position=(32 * r, 0))
            nc.scalar.activation(Pt0[:, 1024:2048], sc0[:, 1024:2048], EXP)
            pair_tiles[0] = Pt0
            vproj(vorder[2])
            vproj(vorder[3])

            for s in range(NST32 + 1):
                qk = 0 < s < NST32
                if qk:
                    # all 4 QK matmuls adjacent: K=32 row-banded matmuls
                    # co-execute in disjoint PE quadrants only when nothing
                    # full-array sits between them.
                    sc = qk_quad(s)
                    Pt = big.tile([P, 2048], BF16, tag="big", name="pt")
                    nc.scalar.activation(Pt[:], sc[:], EXP)
                    pair_tiles[s] = Pt
                for _ in range(5):
                    if wi < len(proj_work):
                        proj_work[wi]()
                        wi += 1
                if s >= 1:
                    s2 = s - 1
                    ci2, g2 = divmod(s2, 8)
                    if g2 == 0:
                        opsA[ci2] = [pav.tile([P, VW], F32, tag="av", name="avo")
                                     for _ in range(2)]
                        for op in opsA[ci2]:
                            nc.vector.memset(op[:], 0.0)
                        if ci2 == 3:
                            opsB[3] = [pp.tile([P, VW], F32, tag="pp", name="avb")
                                       for _ in range(2)]
                            for op in opsB[3]:
                                nc.vector.memset(op[:], 0.0)
                    nq = 4 if ci2 == 3 else 2
                    av_half(s2, [0, 1], nq)
                    av_half(s2, [2, 3], nq)
                    if ci2 == 3:
                        pair_tiles.pop(s2)
                    if g2 == 7:
                        for qs in range(2):
                            av_norm(opsA[ci2][qs], ci2, qs,
                                    act=(ci2 == 3 and qs == 1))
                        del opsA[ci2]
                        if ci2 == 3:
                            av_norm(opsB[3][0], 3, 2)
                            av_norm(opsB[3][1], 3, 3, last=True, act=True)
                            del opsB[3]
                # B-sweeps for chunks 0..2 (after their A sweep ends):
                # chunks 0/1 are emitted in two 4-key-group halves (at the
                # window start and mid) so the second half lands in the
                # otherwise chain-gapped later steps; chunk 2 stays one
                # burst (its pp tiles must be normed before opsB[3]
                # allocates at s=25).
                for c3, s0 in ((0, 9), (1, 17), (2, 24)):
                    half = 8 if c3 == 2 else 2
                    if s == s0 or (c3 != 2 and s in (s0 + 2, s0 + 4, s0 + 6)):
                        g2lo = 0 if s == s0 else 2 * ((s - s0) // 2)
                        if g2lo == 0:
                            opsBd[c3] = [pp.tile([P, VW], F32, tag="pp",
                                                 name="avb")
                                         for _ in range(2)]
                            for op in opsBd[c3]:
                                nc.vector.memset(op[:], 0.0)
                        for g2 in range(g2lo, g2lo + half):
                            m2, t2 = divmod(g2, 4)
                            Pt2 = pair_tiles.pop(8 * c3 + g2)
                            for r in range(4):
                                kt = 16 * m2 + 4 * r + t2
                                for qs in range(2):
                                    nc.tensor.matmul(
                                        opsBd[c3][qs][:],
                                        Pt2[:, 512 * r + P * (qs + 2):512 * r + P * (qs + 3)],
                                        v_sb[kt][:],
                                        start=False, skip_group_check=True,
                                        stop=(g2 == 7 and r == 3))
                        if g2lo + half == 8:
                            for qs in range(2):
                                av_norm(opsBd[c3][qs], c3, qs + 2)
                            del opsBd[c3]

    nc.compile()
    return nc


def _in_maps(x, Wq, bq, Wk, bk, Wv, bv):
    xf = np.ascontiguousarray(np.asarray(x, np.float32).reshape(B, C, HW)).astype(np.float16)
    wpk = np.concatenate([
        np.asarray(Wq, np.float32).T,
        np.asarray(Wk, np.float32).T,
        np.asarray(Wv, np.float32).T], axis=1).astype(np.float16)
    bpk = np.concatenate([
        np.asarray(bq, np.float32).reshape(1, D),
        np.asarray(bk, np.float32).reshape(1, D),
        np.ones((1, PC), np.float32)], axis=1).astype(np.float16)
    maps = []
    for core in range(NCORES):
        b, h = divmod(core, 2)
        xroll = np.concatenate([xf[b][:, h * Q:], xf[b][:, :h * Q]], axis=1)
        maps.append({
            "xkv": np.ascontiguousarray(xroll),
            "wpk": np.ascontiguousarray(wpk),
            "bpk": np.ascontiguousarray(bpk),
        })
    return maps


def _gather(results, bv):
    out = np.empty((B, C, HW), np.float32)
    for core in range(NCORES):
        b, h = divmod(core, 2)
        out[b][:, h * Q:(h + 1) * Q] = results[core]["o"].T
    out += np.asarray(bv, np.float32).reshape(1, C, 1)
    return out.reshape(B, C, H, W)


def run(x, Wq, bq, Wk, bk, Wv, bv, **kwargs):
    with_bias = bool(np.any(np.asarray(bq)) or np.any(np.asarray(bk)))
    key = f"nc{int(with_bias)}"
    nc = _CACHE.get(key)
    if nc is None:
        nc = build_program(with_bias=with_bias)
        _CACHE[key] = nc
    maps = _in_maps(x, Wq, bq, Wk, bk, Wv, bv)
    import concourse.mybir as _mb
    wanted = set()
    for a in nc.m.functions[0].allocations:
        if isinstance(a, _mb.MemoryLocationSet) and a.kind == "ExternalInput":
            wanted.add(a.memorylocations[0].name)
    maps = [{k: v for k, v in m.items() if k in wanted} for m in maps]
    res = run_bass_kernel_spmd(nc, maps, core_ids=list(range(NCORES)), **kwargs)
    return _gather(res.results, bv), res


def kernel(x, Wq, bq, Wk, bk, Wv, bv) -> np.ndarray:
    out, _ = run(x, Wq, bq, Wk, bk, Wv, bv)
    return out


# revision 39
# speedup vs baseline: 1.0146x; 1.0146x over previous
"""Trainium2 Bass kernel for nn_AttentionModule (B=4, C=256, 64x64 spatial).

Reference computation (per batch b, x flattened to [C, HW]):
    q = Wq @ x + bq            [32, HW]
    k = Wk @ x + bk            [32, HW]
    v = x^T @ Wv^T + bv        [HW, 256]
    out = softmax(q^T @ k) @ v [HW, 256] -> transposed to [C, HW]

Sharding: 8 cores, data-parallel over (batch, query-half): core = 2*b + h
computes queries [h*2048, (h+1)*2048) of batch b against all 4096 keys.
Weights replicated.

Numerics: fp16 inputs/projections, fp32 PSUM accumulate, bf16 attention
probabilities (scores reach +-40, exp in fp32 -> bf16, no max-subtraction).

Device layout (v3 — fast prologue + arrival-aware schedule; ~112us vs
v2's ~116.7us in the fast clock state):
  - scores transposed ([keys, q]) so the PE accumulates the softmax
    denominator itself: v carries a ones column, out[:, 256] = sum_k exp.
  - QK is 4-way row-packed: kT4 holds k^T in four 32-partition bands;
    qrep holds q^T replicated at all four bands. Each attention step
    runs 4 adjacent K=32 matmuls (tile_position rows 0/32/64/96) into
    one [128, 2048] psum tile — adjacency matters: K=32 row-banded
    matmuls co-execute in disjoint PE quadrants (~400ns for the quad)
    only when no full-array matmul sits between them and their deps
    are met together. One [128, 2048] EXP per step on ScalarE.
  - steady-state period = quad 400 + sem + EXP 1966 + sem ~= 2580ns
    (single score buffer; a ping-pong split was tried and measured
    equal: it breaks quad co-execution, +240ns/step, while saving the
    same amount of serialization).
  - band layouts are built IN PSUM via tile_position column offsets
    (stationary tile at array cols 32j writes psum partitions
    32j..32j+32), then one DVE copy to SBUF — no SBUF-to-SBUF DMA
    chain on the prologue critical path.
  - split step 0: kproj covers k chunks {0,1} first so QK(0) bands 0,1
    + their EXP half run on x cols 0:1024 while cols 1024:2048 land.
  - AV: P-stationary [q, 258] psum tiles, lag ONE step behind exp;
    A-sweep covers q-subtiles 0,1 inline; B-sweeps for chunks 0/1 are
    emitted as four 2-key-group slices at steps s0/s0+2/s0+4/s0+6
    (s0 = 9/17) so the later slices fill the chain-gapped steps of
    each window; chunk 2 is one burst at s=24 (its pp tiles must be
    normed before opsB[3] allocates at s=25 — finer slicing there
    deadlocks the in-order fetch against the 4-deep wait queue);
    chunk 3 runs all 4 subtiles inline. An 8-way 1-group-per-step
    dribble measured WORSE (+7us) than 2-group slices.
    Normalization = per-partition reciprocal + tensor_scalar multiply
    on VectorE (tail half on ScalarE).
  - input DMA: per-queue transfers serialize at ~75GB/s, so the
    QK(0)-gating 1MB (x cols 0:2048) is spread evenly over the three
    trigger queues (sync/scalar HWDGE + gpsimd SWDGE), first x pieces
    before the tiny wq|wk piece, wv before the back x pieces;
    projection work in the step loop is ordered by DMA arrival.
    NOTE: gpsimd SWDGE x triggers must not lead the program — an early
    SWDGE-first layout correlated with the chip entering a 1.2x-slower
    clock state for the whole run.
  - 8 warm-up matmuls release the HAM clock gate during the DMA window.
  - final [q, c] -> [c, q] transpose + bv bias happen host-side.
  - NOTE: exec time flips between a fast (~112us) and slow (~133us,
    uniform 1.2x on every engine) chip clock state across identical
    invocations; compare only fast-state minima when benchmarking.
"""
import numpy as np
from contextlib import ExitStack

import concourse.bass as bass
import concourse.bacc as bacc
import concourse.tile as tile
from concourse import mybir
from concourse.bass_utils import run_bass_kernel_spmd

B, C, H, W = 4, 256, 64, 64
HW = H * W            # 4096
D = C // 8            # 32 (q/k channels)
NCORES = 8
Q = HW // 2           # 2048 queries per core
P = 128
VW = C + 2            # v tile width (ones col + even-pad)
PC = 512              # projection chunk width

F32 = mybir.dt.float32
F16 = mybir.dt.float16
BF16 = mybir.dt.bfloat16
EXP = mybir.ActivationFunctionType.Exp

_CACHE: dict = {}


def build_program(with_bias: bool = False) -> bacc.Bacc:
    nc = bacc.Bacc("TRN2", target_bir_lowering=False, debug=False)

    # xkv is rolled per-core so the own query half occupies cols [0, 2048):
    # softmax(q k^T) v is invariant to key order, so kT4/v use rolled order too.
    xkv_d = nc.dram_tensor("xkv", [C, HW], F16, kind="ExternalInput").ap()
    # packed weights per c'-half: [wqT | wkT | wvT]
    WB = 2 * D + C        # 320
    wpk_d = nc.dram_tensor("wpk", [C, WB], F16, kind="ExternalInput").ap()
    # packed [bq | bk | ones(PC)]
    bpk_d = nc.dram_tensor("bpk", [1, 2 * D + PC], F16, kind="ExternalInput").ap()
    o_d = nc.dram_tensor("o", [Q, C], F16, kind="ExternalOutput").ap()

    with tile.TileContext(nc) as tc:
        with ExitStack() as ctx:
            big = ctx.enter_context(tc.tile_pool(name="big", bufs=14))
            const = ctx.enter_context(tc.tile_pool(name="const", bufs=1))
            ep = ctx.enter_context(tc.tile_pool(name="ep", bufs=4))
            ps = ctx.enter_context(tc.tile_pool(name="ps", bufs=1, space="PSUM"))
            pav = ctx.enter_context(tc.tile_pool(name="pav", bufs=2, space="PSUM"))
            pp = ctx.enter_context(tc.tile_pool(name="pp", bufs=2, space="PSUM"))

            # ---- PE warm-up: zeroed tile matmuls release the HAM clock
            # gate while the first input DMAs land ----
            dummy = const.tile([P, PC], F16, tag="dummy")
            nc.vector.memset(dummy[:], 0.0)
            for _ in range(8):
                wps = pp.tile([P, PC], F32, tag="pp", name="wps")
                nc.tensor.matmul(wps[:], dummy[:, 0:P], dummy[:],
                                 start=True, stop=True)

            # ---- input DMAs, consumption order, spread over the three
            # trigger queues (sync=SP, scalar=ACT HWDGE; gpsimd SWDGE).
            # Per-queue transfers serialize at ~75GB/s, so the
            # prologue-critical pieces go first on each queue. ----
            wpk_t = [const.tile([P, WB], F16, tag=f"wpk{i}", name=f"wpk{i}")
                     for i in range(2)]
            xkv_t = [const.tile([P, HW], F16, tag=f"xkv{i}", name=f"xkv{i}")
                     for i in range(2)]
            bpk_t = const.tile([1, 2 * D + PC], F16, tag="bpk")

            # The QK(0)-gating input (x cols 0:2048 of both halves, 1MB)
            # is spread evenly over the three trigger queues; per-queue
            # transfers serialize at ~75GB/s. First x pieces precede the
            # tiny wq|wk piece; wv sits before the back x pieces (the
            # first vprojs gate the AV pipeline start).
            eng = [nc.sync, nc.scalar]
            for i in range(2):
                eng[i].dma_start(xkv_t[i][:, 0:512], xkv_d[i * P:(i + 1) * P, 0:512])
            for i in range(2):
                eng[i].dma_start(wpk_t[i][:, 0:2 * D],
                                 wpk_d[i * P:(i + 1) * P, 0:2 * D])
            if with_bias:
                nc.sync.dma_start(bpk_t[:], bpk_d)
            for i in range(2):
                nc.gpsimd.dma_start(xkv_t[i][:, 1536:2048],
                                    xkv_d[i * P:(i + 1) * P, 1536:2048])
            for i in range(2):
                eng[i].dma_start(xkv_t[i][:, 512:1024],
                                 xkv_d[i * P:(i + 1) * P, 512:1024])
            for i in range(2):
                eng[1 - i].dma_start(xkv_t[i][:, 1024:1536],
                                     xkv_d[i * P:(i + 1) * P, 1024:1536])
            for i in range(2):
                eng[i].dma_start(wpk_t[i][:, 2 * D:WB],
                                 wpk_d[i * P:(i + 1) * P, 2 * D:WB])
            for i in range(2):
                eng[i].dma_start(xkv_t[i][:, 2048:3072],
                                 xkv_d[i * P:(i + 1) * P, 2048:3072])
            for i in range(2):
                eng[1 - i].dma_start(xkv_t[i][:, 3072:4096],
                                     xkv_d[i * P:(i + 1) * P, 3072:4096])

            wq_sb = [wpk_t[i][:, 0:D] for i in range(2)]
            wk_sb = [wpk_t[i][:, D:2 * D] for i in range(2)]
            wv_sb = [wpk_t[i][:, 2 * D:WB] for i in range(2)]
            bq_sb = bpk_t[:, 0:D]
            bk_sb = bpk_t[:, D:2 * D]
            ones_sb = bpk_t[:, 2 * D:]

            # kT4: band r (partitions 32r..32r+32) holds k chunks {r, 4+r};
            # QK step (ci, g) with g=(m,t) uses key tiles kt = 16m + 4r + t
            # on band r.
            kT4 = const.tile([P, 2 * PC], F16, tag="kT4")
            # qrep: q^T replicated at all four bands.
            qrep = const.tile([P, Q], F16, tag="qrep")
            v_all = const.tile([P, (HW // P) * VW], F16, tag="vall")
            nc.vector.memset(
                v_all[:].rearrange("p (k c) -> p k c", c=VW)[:, :, C:C + 2], 1.0)
            v_sb = [v_all[:, t * VW:(t + 1) * VW] for t in range(HW // P)]

            # ---- projections (PE, fp16 in / f32 psum) ----
            # Band-packed psum groups: pre-zeroed psum + all-accumulate
            # matmuls with tile_position col offsets (correct under any
            # execution order; the sim's one-group-per-2KB-region check
            # is bypassed).
            def kproj4(m, jlist=None):
                # k chunks 4m..4m+3 -> kT4[:, 512m:512m+512], band layout.
                kp = pp.tile([P, PC], F32, tag="pp", name="kp")
                nc.vector.memset(kp[:], 0.0)
                js = jlist if jlist is not None else range(4 * m, 4 * m + 4)
                mms = []
                for j in js:
                    for h in range(2):
                        mms.append((kp[32 * (j % 4):32 * (j % 4) + 32, :],
                                    wk_sb[h], xkv_t[h][:, PC * j:PC * (j + 1)],
                                    32 * (j % 4)))
                    if with_bias:
                        mms.append((kp[32 * (j % 4):32 * (j % 4) + 32, :],
                                    bk_sb, ones_sb, 32 * (j % 4)))
                for i, (out, lhs, rhs, cp) in enumerate(mms):
                    nc.tensor.matmul(out, lhs, rhs, start=False,
                                     stop=(i == len(mms) - 1),
                                     skip_group_check=True,
                                     tile_position=(0, cp))
                nc.vector.tensor_copy(kT4[:, PC * m:PC * (m + 1)], kp[:])

            def qproj4(j):
                # q chunk j computed 4x via col groups -> psum already
                # band-replicated; one DVE copy, no DMA.
                qp = pp.tile([P, PC], F32, tag="pp", name="qp")
                nc.vector.memset(qp[:], 0.0)
                nmm = 12 if with_bias else 8
                i = 0
                for h in range(2):
                    for r in range(4):
                        nc.tensor.matmul(
                            qp[32 * r:32 * (r + 1), :], wq_sb[h],
                            xkv_t[h][:, PC * j:PC * (j + 1)],
                            start=False, stop=(i == nmm - 1),
                            skip_group_check=True, tile_position=(0, 32 * r))
                        i += 1
                if with_bias:
                    for r in range(4):
                        nc.tensor.matmul(
                            qp[32 * r:32 * (r + 1), :], bq_sb, ones_sb,
                            start=False, stop=(i == nmm - 1),
                            skip_group_check=True, tile_position=(0, 32 * r))
                        i += 1
                nc.vector.tensor_copy(qrep[:, PC * j:PC * (j + 1)], qp[:])

            def qsolo(j):
                # q chunk j (throughput path): 2 matmuls into band 0, then
                # SWDGE replication to bands 1-3 (latency-tolerant).
                qp = pp.tile([D, PC], F32, tag="pp", name="qs")
                nc.tensor.matmul(qp[:], wq_sb[0], xkv_t[0][:, PC * j:PC * (j + 1)],
                                 start=True, stop=False)
                nc.tensor.matmul(qp[:], wq_sb[1], xkv_t[1][:, PC * j:PC * (j + 1)],
                                 start=False, stop=not with_bias)
                if with_bias:
                    nc.tensor.matmul(qp[:], bq_sb, ones_sb, start=False, stop=True)
                nc.vector.tensor_copy(qrep[0:D, PC * j:PC * (j + 1)], qp[:])
                for r in range(1, 4):
                    nc.gpsimd.dma_start(qrep[32 * r:32 * r + 32, PC * j:PC * (j + 1)],
                                        qrep[0:D, PC * j:PC * (j + 1)])

            def vproj(t):
                j, off = divmod(t, PC // P)
                vp = pp.tile([P, C], F32, tag="pp", name="vp")
                for xh in range(2):
                    nc.tensor.matmul(
                        vp[:], xkv_t[xh][:, PC * j + off * P:PC * j + (off + 1) * P],
                        wv_sb[xh], start=(xh == 0), stop=(xh == 1))
                nc.vector.tensor_copy(v_sb[t][:, 0:C], vp[:])

            def kproj2(h2):
                # k chunks {2*h2, 2*h2+1} -> kT4 rows 64*h2:64*h2+64,
                # block 0. Splitting kproj4(0) lets QK(0) bands 0,1 run
                # on x cols 0:1024 while cols 1024:2048 are still landing.
                kp = pp.tile([P, PC], F32, tag="pp", name="kp2")
                rows = slice(64 * h2, 64 * h2 + 64)
                nc.vector.memset(kp[rows, :], 0.0)
                mms = []
                # h2=1: chunk 3 (cols 1536:2048, gpsimd queue) lands
                # before chunk 2 (cols 1024:1536)
                for j in ((3, 2) if h2 == 1 else (0, 1)):
                    for h in range(2):
                        mms.append((kp[32 * j:32 * j + 32, :], wk_sb[h],
                                    xkv_t[h][:, PC * j:PC * (j + 1)], 32 * j))
                    if with_bias:
                        mms.append((kp[32 * j:32 * j + 32, :], bk_sb, ones_sb,
                                    32 * j))
                for i, (out, lhs, rhs, cp) in enumerate(mms):
                    nc.tensor.matmul(out, lhs, rhs, start=False,
                                     stop=(i == len(mms) - 1),
                                     skip_group_check=True,
                                     tile_position=(0, cp))
                nc.vector.tensor_copy(kT4[rows, 0:PC], kp[rows, :])

            # v tiles in AV consumption order (kt = 16m + 4r + t).
            vorder = [16 * m + 4 * r + t
                      for m in range(2) for t in range(4) for r in range(4)]
            # proj_work order tracks x DMA arrival: V4+ read x cols
            # 0:2048 (landed by the loop start); kproj4(1) reads cols
            # 2048:4096 which land ~12us, so it drains at s=2 (used by
            # QK step 4) — emitting it earlier stalls the in-order PE.
            V = [lambda t=t: vproj(t) for t in vorder]
            proj_work = ([lambda: qsolo(1)] + V[4:8]
                         + [lambda: qsolo(2)] + V[8:12]
                         + [lambda: qsolo(3)] + V[12:16]
                         + [lambda: kproj4(1)] + V[16:32])

            # ---- attention: 32 QK steps, AV lags by 1 ----
            def av_norm(op, ci, qb, last=False, act=False):
                rinv = ep.tile([P, 1], F32, tag="rinv", name="rinv")
                nc.vector.reciprocal(rinv[:], op[:, C:C + 1])
                osb = ep.tile([P, C], F16, tag="osb", name="osb")
                if act:
                    # ScalarE is idle after the final exp; runs in parallel
                    # with the DVE half of the last chunk's epilogue.
                    nc.scalar.mul(osb[:], op[:, 0:C], rinv[:])
                else:
                    nc.vector.tensor_scalar_mul(osb[:], op[:, 0:C], rinv[:])
                q0 = ci * 512 + qb * P
                e = nc.scalar if last else nc.sync
                e.dma_start(o_d[q0:q0 + P, :], osb[:])

            pair_tiles = {}
            opsA = {}
            opsB = {}
            wi = 0
            NST32 = 32
            opsBd = {}

            def qk_quad(s):
                ci, g = divmod(s, 8)
                m, t = divmod(g, 4)
                sc = ps.tile([P, 2048], F32, tag="p", name="sc")
                for r in range(4):
                    nc.tensor.matmul(
                        sc[:, 512 * r:512 * (r + 1)],
                        kT4[32 * r:32 * r + 32, PC * m + P * t:PC * m + P * (t + 1)],
                        qrep[32 * r:32 * r + 32, PC * ci:PC * (ci + 1)],
                        start=True, stop=True, tile_position=(32 * r, 0))
                return sc

            def av_half(s2, rlist, nq):
                ci2, g2 = divmod(s2, 8)
                m2, t2 = divmod(g2, 4)
                Pt2 = pair_tiles[s2]
                for r in rlist:
                    kt = 16 * m2 + 4 * r + t2
                    for qs in range(nq):
                        op = opsA[ci2][qs] if qs < 2 else opsB[3][qs - 2]
                        nc.tensor.matmul(
                            op[:],
                            Pt2[:, 512 * r + P * qs:512 * r + P * (qs + 1)],
                            v_sb[kt][:],
                            start=(g2 == 0 and r == 0),
                            stop=(g2 == 7 and r == 3))

            # ---- split step 0: bands 0,1 (k chunks 0,1; x cols 0:1024)
            # run while cols 1024:2048 land; V0-V3 fill the wait. ----
            qproj4(0)
            kproj2(0)
            sc0 = ps.tile([P, 2048], F32, tag="p", name="sc")
            Pt0 = big.tile([P, 2048], BF16, tag="big", name="pt")
            for r in range(2):
                nc.tensor.matmul(
                    sc0[:, 512 * r:512 * (r + 1)],
                    kT4[32 * r:32 * r + 32, 0:P],
                    qrep[32 * r:32 * r + 32, 0:PC],
                    start=True, stop=True, tile_position=(32 * r, 0))
            nc.scalar.activation(Pt0[:, 0:1024], sc0[:, 0:1024], EXP)
            kproj2(1)
            vproj(vorder[0])
            vproj(vorder[1])
            for r in range(2, 4):
                nc.tensor.matmul(
                    sc0[:, 512 * r:512 * (r + 1)],
                    kT4[32 * r:32 * r + 32, 0:P],
                    qrep[32 * r:32 * r + 32, 0:PC],
                    start=True, stop=True, tile_position=(32 * r, 0))
            nc.scalar.activation(Pt0[:, 1024:2048], sc0[:, 1024:2048], EXP)
            pair_tiles[0] = Pt0
            vproj(vorder[2])
            vproj(vorder[3])

            for s in range(NST32 + 1):
                qk = 0 < s < NST32
                if qk:
                    # all 4 QK matmuls adjacent: K=32 row-banded matmuls
                    # co-execute in disjoint PE quadrants only when nothing
                    # full-array sits between them.
                    sc = qk_quad(s)
                    Pt = big.tile([P, 2048], BF16, tag="big", name="pt")
                    nc.scalar.activation(Pt[:], sc[:], EXP)
                    pair_tiles[s] = Pt
                for _ in range(5):
                    if wi < len(proj_work):
                        proj_work[wi]()
                        wi += 1
                if s >= 1:
                    s2 = s - 1
                    ci2, g2 = divmod(s2, 8)
                    if g2 == 0:
                        opsA[ci2] = [pav.tile([P, VW], F32, tag="av", name="avo")
                                     for _ in range(2)]
                        if ci2 == 3:
                            opsB[3] = [pp.tile([P, VW], F32, tag="pp", name="avb")
                                       for _ in range(2)]
                    nq = 4 if ci2 == 3 else 2
                    av_half(s2, [0, 1], nq)
                    av_half(s2, [2, 3], nq)
                    if ci2 == 3:
                        pair_tiles.pop(s2)
                    if g2 == 7:
                        for qs in range(2):
                            av_norm(opsA[ci2][qs], ci2, qs,
                                    act=(ci2 == 3 and qs == 1))
                        del opsA[ci2]
                        if ci2 == 3:
                            av_norm(opsB[3][0], 3, 2)
                            av_norm(opsB[3][1], 3, 3, last=True, act=True)
                            del opsB[3]
                # B-sweeps for chunks 0..2 (after their A sweep ends):
                # chunks 0/1 are emitted in two 4-key-group halves (at the
                # window start and mid) so the second half lands in the
                # otherwise chain-gapped later steps; chunk 2 stays one
                # burst (its pp tiles must be normed before opsB[3]
                # allocates at s=25).
                for c3, s0 in ((0, 9), (1, 17), (2, 24)):
                    half = 8 if c3 == 2 else 2
                    if s == s0 or (c3 != 2 and s in (s0 + 2, s0 + 4, s0 + 6)):
                        g2lo = 0 if s == s0 else 2 * ((s - s0) // 2)
                        if g2lo == 0:
                            opsBd[c3] = [pp.tile([P, VW], F32, tag="pp",
                                                 name="avb")
                                         for _ in range(2)]
                        for g2 in range(g2lo, g2lo + half):
                            m2, t2 = divmod(g2, 4)
                            Pt2 = pair_tiles.pop(8 * c3 + g2)
                            for r in range(4):
                                kt = 16 * m2 + 4 * r + t2
                                for qs in range(2):
                                    nc.tensor.matmul(
                                        opsBd[c3][qs][:],
                                        Pt2[:, 512 * r + P * (qs + 2):512 * r + P * (qs + 3)],
                                        v_sb[kt][:],
                                        start=(g2 == 0 and r == 0),
                                        stop=(g2 == 7 and r == 3))
                        if g2lo + half == 8:
                            for qs in range(2):
                                av_norm(opsBd[c3][qs], c3, qs + 2)
                            del opsBd[c3]

    nc.compile()
    return nc


def _in_maps(x, Wq, bq, Wk, bk, Wv, bv):
    xf = np.ascontiguousarray(np.asarray(x, np.float32).reshape(B, C, HW)).astype(np.float16)
    wpk = np.concatenate([
        np.asarray(Wq, np.float32).T,
        np.asarray(Wk, np.float32).T,
        np.asarray(Wv, np.float32).T], axis=1).astype(np.float16)
    bpk = np.concatenate([
        np.asarray(bq, np.float32).reshape(1, D),
        np.asarray(bk, np.float32).reshape(1, D),
        np.ones((1, PC), np.float32)], axis=1).astype(np.float16)
    maps = []
    for core in range(NCORES):
        b, h = divmod(core, 2)
        xroll = np.concatenate([xf[b][:, h * Q:], xf[b][:, :h * Q]], axis=1)
        maps.append({
            "xkv": np.ascontiguousarray(xroll),
            "wpk": np.ascontiguousarray(wpk),
            "bpk": np.ascontiguousarray(bpk),
        })
    return maps


def _gather(results, bv):
    out = np.empty((B, C, HW), np.float32)
    for core in range(NCORES):
        b, h = divmod(core, 2)
        out[b][:, h * Q:(h + 1) * Q] = results[core]["o"].T
    out += np.asarray(bv, np.float32).reshape(1, C, 1)
    return out.reshape(B, C, H, W)


def run(x, Wq, bq, Wk, bk, Wv, bv, **kwargs):
    with_bias = bool(np.any(np.asarray(bq)) or np.any(np.asarray(bk)))
    key = f"nc{int(with_bias)}"
    nc = _CACHE.get(key)
    if nc is None:
        nc = build_program(with_bias=with_bias)
        _CACHE[key] = nc
    maps = _in_maps(x, Wq, bq, Wk, bk, Wv, bv)
    import concourse.mybir as _mb
    wanted = set()
    for a in nc.m.functions[0].allocations:
        if isinstance(a, _mb.MemoryLocationSet) and a.kind == "ExternalInput":
            wanted.add(a.memorylocations[0].name)
    maps = [{k: v for k, v in m.items() if k in wanted} for m in maps]
    res = run_bass_kernel_spmd(nc, maps, core_ids=list(range(NCORES)), **kwargs)
    return _gather(res.results, bv), res


def kernel(x, Wq, bq, Wk, bk, Wv, bv) -> np.ndarray:
    out, _ = run(x, Wq, bq, Wk, bk, Wv, bv)
    return out


# revision 41
# speedup vs baseline: 1.0359x; 1.0210x over previous
"""Trainium2 Bass kernel for nn_AttentionModule (B=4, C=256, 64x64 spatial).

Reference computation (per batch b, x flattened to [C, HW]):
    q = Wq @ x + bq            [32, HW]
    k = Wk @ x + bk            [32, HW]
    v = x^T @ Wv^T + bv        [HW, 256]
    out = softmax(q^T @ k) @ v [HW, 256] -> transposed to [C, HW]

Sharding: 8 cores, data-parallel over (batch, query-half): core = 2*b + h
computes queries [h*2048, (h+1)*2048) of batch b against all 4096 keys.
Weights replicated.

Numerics: fp16 inputs/projections, fp32 PSUM accumulate, bf16 attention
probabilities (scores reach +-40, exp in fp32 -> bf16, no max-subtraction).

Device layout (v3 — fast prologue + arrival-aware schedule; ~112us vs
v2's ~116.7us in the fast clock state):
  - scores transposed ([keys, q]) so the PE accumulates the softmax
    denominator itself: v carries a ones column, out[:, 256] = sum_k exp.
  - QK is 4-way row-packed: kT4 holds k^T in four 32-partition bands;
    qrep holds q^T replicated at all four bands. Each attention step
    runs 4 adjacent K=32 matmuls (tile_position rows 0/32/64/96) into
    one [128, 2048] psum tile — adjacency matters: K=32 row-banded
    matmuls co-execute in disjoint PE quadrants (~400ns for the quad)
    only when no full-array matmul sits between them and their deps
    are met together. One [128, 2048] EXP per step on ScalarE.
  - steady-state period = quad 400 + sem + EXP 1966 + sem ~= 2580ns
    (single score buffer; a ping-pong split was tried and measured
    equal: it breaks quad co-execution, +240ns/step, while saving the
    same amount of serialization).
  - band layouts are built IN PSUM via tile_position column offsets
    (stationary tile at array cols 32j writes psum partitions
    32j..32j+32), then one DVE copy to SBUF — no SBUF-to-SBUF DMA
    chain on the prologue critical path.
  - split step 0: kproj covers k chunks {0,1} first so QK(0) bands 0,1
    + their EXP half run on x cols 0:1024 while cols 1024:2048 land.
  - AV: P-stationary [q, 258] psum tiles, lag ONE step behind exp;
    A-sweep covers q-subtiles 0,1 inline; B-sweeps for chunks 0/1 are
    emitted as four 2-key-group slices at steps s0/s0+2/s0+4/s0+6
    (s0 = 9/17) so the later slices fill the chain-gapped steps of
    each window; chunk 2 is one burst at s=24 (its pp tiles must be
    normed before opsB[3] allocates at s=25 — finer slicing there
    deadlocks the in-order fetch against the 4-deep wait queue);
    chunk 3 runs all 4 subtiles inline. An 8-way 1-group-per-step
    dribble measured WORSE (+7us) than 2-group slices.
    Normalization = per-partition reciprocal + tensor_scalar multiply
    on VectorE (tail half on ScalarE).
  - input DMA: per-queue transfers serialize at ~75GB/s, so the
    QK(0)-gating 1MB (x cols 0:2048) is spread evenly over the three
    trigger queues (sync/scalar HWDGE + gpsimd SWDGE), first x pieces
    before the tiny wq|wk piece, wv before the back x pieces;
    projection work in the step loop is ordered by DMA arrival.
    NOTE: gpsimd SWDGE x triggers must not lead the program — an early
    SWDGE-first layout correlated with the chip entering a 1.2x-slower
    clock state for the whole run.
  - 8 warm-up matmuls release the HAM clock gate during the DMA window.
  - final [q, c] -> [c, q] transpose + bv bias happen host-side.
  - NOTE: exec time flips between a fast (~112us) and slow (~133us,
    uniform 1.2x on every engine) chip clock state across identical
    invocations; compare only fast-state minima when benchmarking.
"""
import numpy as np
from contextlib import ExitStack

import concourse.bass as bass
import concourse.bacc as bacc
import concourse.tile as tile
from concourse import mybir
from concourse.bass_utils import run_bass_kernel_spmd

B, C, H, W = 4, 256, 64, 64
HW = H * W            # 4096
D = C // 8            # 32 (q/k channels)
NCORES = 8
Q = HW // 2           # 2048 queries per core
P = 128
VW = C + 2            # v tile width (ones col + even-pad)
PC = 512              # projection chunk width

F32 = mybir.dt.float32
F16 = mybir.dt.float16
BF16 = mybir.dt.bfloat16
EXP = mybir.ActivationFunctionType.Exp

_CACHE: dict = {}


def build_program(with_bias: bool = False) -> bacc.Bacc:
    nc = bacc.Bacc("TRN2", target_bir_lowering=False, debug=False)

    # xkv is rolled per-core so the own query half occupies cols [0, 2048):
    # softmax(q k^T) v is invariant to key order, so kT4/v use rolled order too.
    xkv_d = nc.dram_tensor("xkv", [C, HW], F16, kind="ExternalInput").ap()
    # packed weights per c'-half: [wqT | wkT | wvT]
    WB = 2 * D + C        # 320
    wpk_d = nc.dram_tensor("wpk", [C, WB], F16, kind="ExternalInput").ap()
    # packed [bq | bk | ones(PC)]
    bpk_d = nc.dram_tensor("bpk", [1, 2 * D + PC], F16, kind="ExternalInput").ap()
    o_d = nc.dram_tensor("o", [Q, C], F16, kind="ExternalOutput").ap()

    with tile.TileContext(nc) as tc:
        with ExitStack() as ctx:
            big = ctx.enter_context(tc.tile_pool(name="big", bufs=14))
            const = ctx.enter_context(tc.tile_pool(name="const", bufs=1))
            ep = ctx.enter_context(tc.tile_pool(name="ep", bufs=4))
            ps = ctx.enter_context(tc.tile_pool(name="ps", bufs=1, space="PSUM"))
            pav = ctx.enter_context(tc.tile_pool(name="pav", bufs=2, space="PSUM"))
            pp = ctx.enter_context(tc.tile_pool(name="pp", bufs=2, space="PSUM"))

            # ---- PE warm-up: zeroed tile matmuls release the HAM clock
            # gate while the first input DMAs land ----
            dummy = const.tile([P, PC], F16, tag="dummy")
            nc.vector.memset(dummy[:], 0.0)
            for _ in range(8):
                wps = pp.tile([P, PC], F32, tag="pp", name="wps")
                nc.tensor.matmul(wps[:], dummy[:, 0:P], dummy[:],
                                 start=True, stop=True)

            # ---- input DMAs, consumption order, spread over the three
            # trigger queues (sync=SP, scalar=ACT HWDGE; gpsimd SWDGE).
            # Per-queue transfers serialize at ~75GB/s, so the
            # prologue-critical pieces go first on each queue. ----
            wpk_t = [const.tile([P, WB], F16, tag=f"wpk{i}", name=f"wpk{i}")
                     for i in range(2)]
            xkv_t = [const.tile([P, HW], F16, tag=f"xkv{i}", name=f"xkv{i}")
                     for i in range(2)]
            bpk_t = const.tile([1, 2 * D + PC], F16, tag="bpk")

            # The QK(0)-gating input (x cols 0:2048 of both halves, 1MB)
            # is spread evenly over the three trigger queues; per-queue
            # transfers serialize at ~75GB/s. First x pieces precede the
            # tiny wq|wk piece; wv sits before the back x pieces (the
            # first vprojs gate the AV pipeline start).
            eng = [nc.sync, nc.scalar]
            for i in range(2):
                eng[i].dma_start(xkv_t[i][:, 0:512], xkv_d[i * P:(i + 1) * P, 0:512])
            for i in range(2):
                eng[i].dma_start(wpk_t[i][:, 0:2 * D],
                                 wpk_d[i * P:(i + 1) * P, 0:2 * D])
            if with_bias:
                nc.sync.dma_start(bpk_t[:], bpk_d)
            for i in range(2):
                nc.gpsimd.dma_start(xkv_t[i][:, 1024:1536],
                                    xkv_d[i * P:(i + 1) * P, 1024:1536])
            for i in range(2):
                eng[i].dma_start(xkv_t[i][:, 512:1024],
                                 xkv_d[i * P:(i + 1) * P, 512:1024])
            for i in range(2):
                eng[1 - i].dma_start(xkv_t[i][:, 1536:2048],
                                     xkv_d[i * P:(i + 1) * P, 1536:2048])
            for i in range(2):
                eng[i].dma_start(wpk_t[i][:, 2 * D:WB],
                                 wpk_d[i * P:(i + 1) * P, 2 * D:WB])
            for i in range(2):
                eng[i].dma_start(xkv_t[i][:, 2048:3072],
                                 xkv_d[i * P:(i + 1) * P, 2048:3072])
            for i in range(2):
                eng[1 - i].dma_start(xkv_t[i][:, 3072:4096],
                                     xkv_d[i * P:(i + 1) * P, 3072:4096])

            wq_sb = [wpk_t[i][:, 0:D] for i in range(2)]
            wk_sb = [wpk_t[i][:, D:2 * D] for i in range(2)]
            wv_sb = [wpk_t[i][:, 2 * D:WB] for i in range(2)]
            bq_sb = bpk_t[:, 0:D]
            bk_sb = bpk_t[:, D:2 * D]
            ones_sb = bpk_t[:, 2 * D:]

            # kT4: band r (partitions 32r..32r+32) holds k chunks {r, 4+r};
            # QK step (ci, g) with g=(m,t) uses key tiles kt = 16m + 4r + t
            # on band r.
            kT4 = const.tile([P, 2 * PC], F16, tag="kT4")
            # qrep: q^T replicated at all four bands.
            qrep = const.tile([P, Q], F16, tag="qrep")
            v_all = const.tile([P, (HW // P) * VW], F16, tag="vall")
            nc.vector.memset(
                v_all[:].rearrange("p (k c) -> p k c", c=VW)[:, :, C:C + 2], 1.0)
            v_sb = [v_all[:, t * VW:(t + 1) * VW] for t in range(HW // P)]

            # ---- projections (PE, fp16 in / f32 psum) ----
            # Band-packed psum groups: pre-zeroed psum + all-accumulate
            # matmuls with tile_position col offsets (correct under any
            # execution order; the sim's one-group-per-2KB-region check
            # is bypassed).
            def kproj256(m4):
                # 256-col k chunks c = 4*m4+r -> kT4[:, 256*m4:+256], band
                # layout. 256-col granularity halves the QK(0)-gating input
                # (block 0 = x cols 0:1024) vs 512-col chunks.
                kp = pp.tile([P, 256], F32, tag="pp", name="kp")
                nc.vector.memset(kp[:], 0.0)
                mms = []
                for r in range(4):
                    c = 4 * m4 + r
                    for h in range(2):
                        mms.append((kp[32 * r:32 * r + 32, :], wk_sb[h],
                                    xkv_t[h][:, 256 * c:256 * (c + 1)], 32 * r))
                    if with_bias:
                        mms.append((kp[32 * r:32 * r + 32, :], bk_sb,
                                    ones_sb[:, 0:256], 32 * r))
                for i, (out, lhs, rhs, cp) in enumerate(mms):
                    nc.tensor.matmul(out, lhs, rhs, start=False,
                                     stop=(i == len(mms) - 1),
                                     skip_group_check=True,
                                     tile_position=(0, cp))
                nc.vector.tensor_copy(kT4[:, 256 * m4:256 * (m4 + 1)], kp[:])

            def qproj4(j):
                # q chunk j computed 4x via col groups -> psum already
                # band-replicated; one DVE copy, no DMA.
                qp = pp.tile([P, PC], F32, tag="pp", name="qp")
                nc.vector.memset(qp[:], 0.0)
                nmm = 12 if with_bias else 8
                i = 0
                for h in range(2):
                    for r in range(4):
                        nc.tensor.matmul(
                            qp[32 * r:32 * (r + 1), :], wq_sb[h],
                            xkv_t[h][:, PC * j:PC * (j + 1)],
                            start=False, stop=(i == nmm - 1),
                            skip_group_check=True, tile_position=(0, 32 * r))
                        i += 1
                if with_bias:
                    for r in range(4):
                        nc.tensor.matmul(
                            qp[32 * r:32 * (r + 1), :], bq_sb, ones_sb,
                            start=False, stop=(i == nmm - 1),
                            skip_group_check=True, tile_position=(0, 32 * r))
                        i += 1
                nc.vector.tensor_copy(qrep[:, PC * j:PC * (j + 1)], qp[:])

            def qsolo(j):
                # q chunk j (throughput path): 2 matmuls into band 0, then
                # SWDGE replication to bands 1-3 (latency-tolerant).
                qp = pp.tile([D, PC], F32, tag="pp", name="qs")
                nc.tensor.matmul(qp[:], wq_sb[0], xkv_t[0][:, PC * j:PC * (j + 1)],
                                 start=True, stop=False)
                nc.tensor.matmul(qp[:], wq_sb[1], xkv_t[1][:, PC * j:PC * (j + 1)],
                                 start=False, stop=not with_bias)
                if with_bias:
                    nc.tensor.matmul(qp[:], bq_sb, ones_sb, start=False, stop=True)
                nc.vector.tensor_copy(qrep[0:D, PC * j:PC * (j + 1)], qp[:])
                for r in range(1, 4):
                    nc.gpsimd.dma_start(qrep[32 * r:32 * r + 32, PC * j:PC * (j + 1)],
                                        qrep[0:D, PC * j:PC * (j + 1)])

            def vproj(t):
                j, off = divmod(t, PC // P)
                vp = pp.tile([P, C], F32, tag="pp", name="vp")
                for xh in range(2):
                    nc.tensor.matmul(
                        vp[:], xkv_t[xh][:, PC * j + off * P:PC * j + (off + 1) * P],
                        wv_sb[xh], start=(xh == 0), stop=(xh == 1))
                nc.vector.tensor_copy(v_sb[t][:, 0:C], vp[:])

            # v tiles in AV consumption order: step group g=(m4,t) uses
            # key tile kt = 2*(4*(g//2)+r) + g%2 on band r.
            def kt_of(g, r):
                return 2 * (4 * (g // 2) + r) + (g % 2)
            vorder = [kt_of(g, r) for g in range(8) for r in range(4)]
            V = [lambda t=t: vproj(t) for t in vorder]
            proj_work = ([lambda: kproj256(1), lambda: qsolo(1)] + V[0:3]
                         + V[3:4] + [lambda: qsolo(2)] + V[4:7]
                         + [lambda: kproj256(2), lambda: qsolo(3)] + V[7:10]
                         + V[10:15]
                         + [lambda: kproj256(3)] + V[15:19]
                         + V[19:32])

            # ---- attention: 32 QK steps, AV lags by 1 ----
            def av_norm(op, ci, qb, last=False, act=False):
                rinv = ep.tile([P, 1], F32, tag="rinv", name="rinv")
                nc.vector.reciprocal(rinv[:], op[:, C:C + 1])
                osb = ep.tile([P, C], F16, tag="osb", name="osb")
                if act:
                    # ScalarE is idle after the final exp; runs in parallel
                    # with the DVE half of the last chunk's epilogue.
                    nc.scalar.mul(osb[:], op[:, 0:C], rinv[:])
                else:
                    nc.vector.tensor_scalar_mul(osb[:], op[:, 0:C], rinv[:])
                q0 = ci * 512 + qb * P
                e = nc.scalar if last else nc.sync
                e.dma_start(o_d[q0:q0 + P, :], osb[:])

            pair_tiles = {}
            opsA = {}
            opsB = {}
            wi = 0
            NST32 = 32
            opsBd = {}

            def qk_quad(s):
                ci, g = divmod(s, 8)
                m4g, t = divmod(g, 2)
                sc = ps.tile([P, 2048], F32, tag="p", name="sc")
                for r in range(4):
                    nc.tensor.matmul(
                        sc[:, 512 * r:512 * (r + 1)],
                        kT4[32 * r:32 * r + 32, 256 * m4g + P * t:256 * m4g + P * (t + 1)],
                        qrep[32 * r:32 * r + 32, PC * ci:PC * (ci + 1)],
                        start=True, stop=True, tile_position=(32 * r, 0))
                return sc

            def av_half(s2, rlist, nq):
                ci2, g2 = divmod(s2, 8)
                Pt2 = pair_tiles[s2]
                for r in rlist:
                    kt = kt_of(g2, r)
                    for qs in range(nq):
                        op = opsA[ci2][qs] if qs < 2 else opsB[3][qs - 2]
                        nc.tensor.matmul(
                            op[:],
                            Pt2[:, 512 * r + P * qs:512 * r + P * (qs + 1)],
                            v_sb[kt][:],
                            start=(g2 == 0 and r == 0),
                            stop=(g2 == 7 and r == 3))

            # ---- prologue: QK(0) needs only k block 0 (x cols 0:1024)
            # and q chunk 0; the uniform loop starts at s=0 ----
            qproj4(0)
            kproj256(0)

            for s in range(NST32 + 1):
                qk = s < NST32
                if qk:
                    # all 4 QK matmuls adjacent: K=32 row-banded matmuls
                    # co-execute in disjoint PE quadrants only when nothing
                    # full-array sits between them.
                    sc = qk_quad(s)
                    Pt = big.tile([P, 2048], BF16, tag="big", name="pt")
                    nc.scalar.activation(Pt[:], sc[:], EXP)
                    pair_tiles[s] = Pt
                for _ in range(5):
                    if wi < len(proj_work):
                        proj_work[wi]()
                        wi += 1
                if s >= 1:
                    s2 = s - 1
                    ci2, g2 = divmod(s2, 8)
                    if g2 == 0:
                        opsA[ci2] = [pav.tile([P, VW], F32, tag="av", name="avo")
                                     for _ in range(2)]
                        if ci2 == 3:
                            opsB[3] = [pp.tile([P, VW], F32, tag="pp", name="avb")
                                       for _ in range(2)]
                    nq = 4 if ci2 == 3 else 2
                    av_half(s2, [0, 1], nq)
                    av_half(s2, [2, 3], nq)
                    if ci2 == 3:
                        pair_tiles.pop(s2)
                    if g2 == 7:
                        for qs in range(2):
                            av_norm(opsA[ci2][qs], ci2, qs,
                                    act=(ci2 == 3 and qs == 1))
                        del opsA[ci2]
                        if ci2 == 3:
                            av_norm(opsB[3][0], 3, 2)
                            av_norm(opsB[3][1], 3, 3, last=True, act=True)
                            del opsB[3]
                # B-sweeps for chunks 0..2 (after their A sweep ends):
                # chunks 0/1 are emitted in two 4-key-group halves (at the
                # window start and mid) so the second half lands in the
                # otherwise chain-gapped later steps; chunk 2 stays one
                # burst (its pp tiles must be normed before opsB[3]
                # allocates at s=25).
                for c3, s0 in ((0, 9), (1, 17), (2, 24)):
                    half = 8 if c3 == 2 else 2
                    if s == s0 or (c3 != 2 and s in (s0 + 2, s0 + 4, s0 + 6)):
                        g2lo = 0 if s == s0 else 2 * ((s - s0) // 2)
                        if g2lo == 0:
                            opsBd[c3] = [pp.tile([P, VW], F32, tag="pp",
                                                 name="avb")
                                         for _ in range(2)]
                        for g2 in range(g2lo, g2lo + half):
                            Pt2 = pair_tiles.pop(8 * c3 + g2)
                            for r in range(4):
                                kt = kt_of(g2, r)
                                for qs in range(2):
                                    nc.tensor.matmul(
                                        opsBd[c3][qs][:],
                                        Pt2[:, 512 * r + P * (qs + 2):512 * r + P * (qs + 3)],
                                        v_sb[kt][:],
                                        start=(g2 == 0 and r == 0),
                                        stop=(g2 == 7 and r == 3))
                        if g2lo + half == 8:
                            for qs in range(2):
                                av_norm(opsBd[c3][qs], c3, qs + 2)
                            del opsBd[c3]

    nc.compile()
    return nc


def _in_maps(x, Wq, bq, Wk, bk, Wv, bv):
    xf = np.ascontiguousarray(np.asarray(x, np.float32).reshape(B, C, HW)).astype(np.float16)
    wpk = np.concatenate([
        np.asarray(Wq, np.float32).T,
        np.asarray(Wk, np.float32).T,
        np.asarray(Wv, np.float32).T], axis=1).astype(np.float16)
    bpk = np.concatenate([
        np.asarray(bq, np.float32).reshape(1, D),
        np.asarray(bk, np.float32).reshape(1, D),
        np.ones((1, PC), np.float32)], axis=1).astype(np.float16)
    maps = []
    for core in range(NCORES):
        b, h = divmod(core, 2)
        xroll = np.concatenate([xf[b][:, h * Q:], xf[b][:, :h * Q]], axis=1)
        maps.append({
            "xkv": np.ascontiguousarray(xroll),
            "wpk": np.ascontiguousarray(wpk),
            "bpk": np.ascontiguousarray(bpk),
        })
    return maps


def _gather(results, bv):
    out = np.empty((B, C, HW), np.float32)
    for core in range(NCORES):
        b, h = divmod(core, 2)
        out[b][:, h * Q:(h + 1) * Q] = results[core]["o"].T
    out += np.asarray(bv, np.float32).reshape(1, C, 1)
    return out.reshape(B, C, H, W)


def run(x, Wq, bq, Wk, bk, Wv, bv, **kwargs):
    with_bias = bool(np.any(np.asarray(bq)) or np.any(np.asarray(bk)))
    key = f"nc{int(with_bias)}"
    nc = _CACHE.get(key)
    if nc is None:
        nc = build_program(with_bias=with_bias)
        _CACHE[key] = nc
    maps = _in_maps(x, Wq, bq, Wk, bk, Wv, bv)
    import concourse.mybir as _mb
    wanted = set()
    for a in nc.m.functions[0].allocations:
        if isinstance(a, _mb.MemoryLocationSet) and a.kind == "ExternalInput":
            wanted.add(a.memorylocations[0].name)
    maps = [{k: v for k, v in m.items() if k in wanted} for m in maps]
    res = run_bass_kernel_spmd(nc, maps, core_ids=list(range(NCORES)), **kwargs)
    return _gather(res.results, bv), res


def kernel(x, Wq, bq, Wk, bk, Wv, bv) -> np.ndarray:
    out, _ = run(x, Wq, bq, Wk, bk, Wv, bv)
    return out


# revision 42
# speedup vs baseline: 1.0377x; 1.0017x over previous
"""Trainium2 Bass kernel for nn_AttentionModule (B=4, C=256, 64x64 spatial).

Reference computation (per batch b, x flattened to [C, HW]):
    q = Wq @ x + bq            [32, HW]
    k = Wk @ x + bk            [32, HW]
    v = x^T @ Wv^T + bv        [HW, 256]
    out = softmax(q^T @ k) @ v [HW, 256] -> transposed to [C, HW]

Sharding: 8 cores, data-parallel over (batch, query-half): core = 2*b + h
computes queries [h*2048, (h+1)*2048) of batch b against all 4096 keys.
Weights replicated.

Numerics: fp16 inputs/projections, fp32 PSUM accumulate, bf16 attention
probabilities (scores reach +-40, exp in fp32 -> bf16, no max-subtraction).

Device layout (v3 — fast prologue + arrival-aware schedule; ~112us vs
v2's ~116.7us in the fast clock state):
  - scores transposed ([keys, q]) so the PE accumulates the softmax
    denominator itself: v carries a ones column, out[:, 256] = sum_k exp.
  - QK is 4-way row-packed: kT4 holds k^T in four 32-partition bands;
    qrep holds q^T replicated at all four bands. Each attention step
    runs 4 adjacent K=32 matmuls (tile_position rows 0/32/64/96) into
    one [128, 2048] psum tile — adjacency matters: K=32 row-banded
    matmuls co-execute in disjoint PE quadrants (~400ns for the quad)
    only when no full-array matmul sits between them and their deps
    are met together. One [128, 2048] EXP per step on ScalarE.
  - steady-state period = quad 400 + sem + EXP 1966 + sem ~= 2580ns
    (single score buffer; a ping-pong split was tried and measured
    equal: it breaks quad co-execution, +240ns/step, while saving the
    same amount of serialization).
  - band layouts are built IN PSUM via tile_position column offsets
    (stationary tile at array cols 32j writes psum partitions
    32j..32j+32), then one DVE copy to SBUF — no SBUF-to-SBUF DMA
    chain on the prologue critical path.
  - k-projection chunks are 256 cols (kproj256 group m4 packs chunks
    4*m4+r into band r): kT4 block 0 then needs only x cols 0:1024
    (0.5MB) instead of 0:2048, so QK(0) starts ~2us earlier and every
    later k-block/v-tile deadline gets one extra block of DMA slack
    (key tile kt = 2*(4*(g//2)+r) + g%2 for step group g, band r).
  - AV: P-stationary [q, 258] psum tiles, lag ONE step behind exp;
    A-sweep covers q-subtiles 0,1 inline; B-sweeps for chunks 0/1 are
    emitted as four 2-key-group slices at steps s0/s0+2/s0+4/s0+6
    (s0 = 9/17) so the later slices fill the chain-gapped steps of
    each window; chunk 2 is one burst at s=24 (its pp tiles must be
    normed before opsB[3] allocates at s=25 — finer slicing there
    deadlocks the in-order fetch against the 4-deep wait queue);
    chunk 3 runs all 4 subtiles inline. An 8-way 1-group-per-step
    dribble measured WORSE (+7us) than 2-group slices.
    Normalization = per-partition reciprocal + tensor_scalar multiply
    on VectorE (tail half on ScalarE).
  - input DMA: per-queue transfers serialize at ~75GB/s; the front
    (x cols 0:1024) rides the two HWDGE queues, cols 1024:1536 go via
    gpsimd SWDGE, first x pieces before the tiny wq|wk piece, wv
    before the back x pieces;
    projection work in the step loop is ordered by DMA arrival.
    NOTE: gpsimd SWDGE x triggers must not lead the program — an early
    SWDGE-first layout correlated with the chip entering a 1.2x-slower
    clock state for the whole run.
  - 8 warm-up matmuls release the HAM clock gate during the DMA window.
  - final [q, c] -> [c, q] transpose + bv bias happen host-side.
  - NOTE: exec time flips between a fast (~112us) and slow (~133us,
    uniform 1.2x on every engine) chip clock state across identical
    invocations; compare only fast-state minima when benchmarking.
"""
import numpy as np
from contextlib import ExitStack

import concourse.bass as bass
import concourse.bacc as bacc
import concourse.tile as tile
from concourse import mybir
from concourse.bass_utils import run_bass_kernel_spmd

B, C, H, W = 4, 256, 64, 64
HW = H * W            # 4096
D = C // 8            # 32 (q/k channels)
NCORES = 8
Q = HW // 2           # 2048 queries per core
P = 128
VW = C + 2            # v tile width (ones col + even-pad)
PC = 512              # projection chunk width

F32 = mybir.dt.float32
F16 = mybir.dt.float16
BF16 = mybir.dt.bfloat16
EXP = mybir.ActivationFunctionType.Exp

_CACHE: dict = {}


def build_program(with_bias: bool = False) -> bacc.Bacc:
    nc = bacc.Bacc("TRN2", target_bir_lowering=False, debug=False)

    # xkv is rolled per-core so the own query half occupies cols [0, 2048):
    # softmax(q k^T) v is invariant to key order, so kT4/v use rolled order too.
    xkv_d = nc.dram_tensor("xkv", [C, HW], F16, kind="ExternalInput").ap()
    # packed weights per c'-half: [wqT | wkT | wvT]
    WB = 2 * D + C        # 320
    wpk_d = nc.dram_tensor("wpk", [C, WB], F16, kind="ExternalInput").ap()
    # packed [bq | bk | ones(PC)]
    bpk_d = nc.dram_tensor("bpk", [1, 2 * D + PC], F16, kind="ExternalInput").ap()
    o_d = nc.dram_tensor("o", [Q, C], F16, kind="ExternalOutput").ap()

    with tile.TileContext(nc) as tc:
        with ExitStack() as ctx:
            big = ctx.enter_context(tc.tile_pool(name="big", bufs=14))
            const = ctx.enter_context(tc.tile_pool(name="const", bufs=1))
            ep = ctx.enter_context(tc.tile_pool(name="ep", bufs=4))
            ps = ctx.enter_context(tc.tile_pool(name="ps", bufs=1, space="PSUM"))
            pav = ctx.enter_context(tc.tile_pool(name="pav", bufs=2, space="PSUM"))
            pp = ctx.enter_context(tc.tile_pool(name="pp", bufs=2, space="PSUM"))

            # ---- PE warm-up: zeroed tile matmuls release the HAM clock
            # gate while the first input DMAs land ----
            dummy = const.tile([P, PC], F16, tag="dummy")
            nc.vector.memset(dummy[:], 0.0)
            for _ in range(8):
                wps = pp.tile([P, PC], F32, tag="pp", name="wps")
                nc.tensor.matmul(wps[:], dummy[:, 0:P], dummy[:],
                                 start=True, stop=True)

            # ---- input DMAs, consumption order, spread over the three
            # trigger queues (sync=SP, scalar=ACT HWDGE; gpsimd SWDGE).
            # Per-queue transfers serialize at ~75GB/s, so the
            # prologue-critical pieces go first on each queue. ----
            wpk_t = [const.tile([P, WB], F16, tag=f"wpk{i}", name=f"wpk{i}")
                     for i in range(2)]
            xkv_t = [const.tile([P, HW], F16, tag=f"xkv{i}", name=f"xkv{i}")
                     for i in range(2)]
            bpk_t = const.tile([1, 2 * D + PC], F16, tag="bpk")

            # The QK(0)-gating input (x cols 0:2048 of both halves, 1MB)
            # is spread evenly over the three trigger queues; per-queue
            # transfers serialize at ~75GB/s. First x pieces precede the
            # tiny wq|wk piece; wv sits before the back x pieces (the
            # first vprojs gate the AV pipeline start).
            eng = [nc.sync, nc.scalar]
            for i in range(2):
                eng[i].dma_start(xkv_t[i][:, 0:512], xkv_d[i * P:(i + 1) * P, 0:512])
            for i in range(2):
                eng[i].dma_start(wpk_t[i][:, 0:2 * D],
                                 wpk_d[i * P:(i + 1) * P, 0:2 * D])
            if with_bias:
                nc.sync.dma_start(bpk_t[:], bpk_d)
            for i in range(2):
                nc.gpsimd.dma_start(xkv_t[i][:, 1024:1536],
                                    xkv_d[i * P:(i + 1) * P, 1024:1536])
            for i in range(2):
                eng[i].dma_start(xkv_t[i][:, 512:1024],
                                 xkv_d[i * P:(i + 1) * P, 512:1024])
            for i in range(2):
                eng[1 - i].dma_start(xkv_t[i][:, 1536:2048],
                                     xkv_d[i * P:(i + 1) * P, 1536:2048])
            for i in range(2):
                eng[i].dma_start(wpk_t[i][:, 2 * D:WB],
                                 wpk_d[i * P:(i + 1) * P, 2 * D:WB])
            for i in range(2):
                eng[i].dma_start(xkv_t[i][:, 2048:3072],
                                 xkv_d[i * P:(i + 1) * P, 2048:3072])
            for i in range(2):
                eng[1 - i].dma_start(xkv_t[i][:, 3072:4096],
                                     xkv_d[i * P:(i + 1) * P, 3072:4096])

            wq_sb = [wpk_t[i][:, 0:D] for i in range(2)]
            wk_sb = [wpk_t[i][:, D:2 * D] for i in range(2)]
            wv_sb = [wpk_t[i][:, 2 * D:WB] for i in range(2)]
            bq_sb = bpk_t[:, 0:D]
            bk_sb = bpk_t[:, D:2 * D]
            ones_sb = bpk_t[:, 2 * D:]

            # kT4: band r (partitions 32r..32r+32) holds k chunks {r, 4+r};
            # QK step (ci, g) with g=(m,t) uses key tiles kt = 16m + 4r + t
            # on band r.
            kT4 = const.tile([P, 2 * PC], F16, tag="kT4")
            # qrep: q^T replicated at all four bands.
            qrep = const.tile([P, Q], F16, tag="qrep")
            v_all = const.tile([P, (HW // P) * VW], F16, tag="vall")
            nc.vector.memset(
                v_all[:].rearrange("p (k c) -> p k c", c=VW)[:, :, C:C + 2], 1.0)
            v_sb = [v_all[:, t * VW:(t + 1) * VW] for t in range(HW // P)]

            # ---- projections (PE, fp16 in / f32 psum) ----
            # Band-packed psum groups: pre-zeroed psum + all-accumulate
            # matmuls with tile_position col offsets (correct under any
            # execution order; the sim's one-group-per-2KB-region check
            # is bypassed).
            def kproj256(m4):
                # 256-col k chunks c = 4*m4+r -> kT4[:, 256*m4:+256], band
                # layout. 256-col granularity halves the QK(0)-gating input
                # (block 0 = x cols 0:1024) vs 512-col chunks.
                kp = pp.tile([P, 256], F32, tag="pp", name="kp")
                nc.vector.memset(kp[:], 0.0)
                mms = []
                for r in range(4):
                    c = 4 * m4 + r
                    for h in range(2):
                        mms.append((kp[32 * r:32 * r + 32, :], wk_sb[h],
                                    xkv_t[h][:, 256 * c:256 * (c + 1)], 32 * r))
                    if with_bias:
                        mms.append((kp[32 * r:32 * r + 32, :], bk_sb,
                                    ones_sb[:, 0:256], 32 * r))
                for i, (out, lhs, rhs, cp) in enumerate(mms):
                    nc.tensor.matmul(out, lhs, rhs, start=False,
                                     stop=(i == len(mms) - 1),
                                     skip_group_check=True,
                                     tile_position=(0, cp))
                nc.vector.tensor_copy(kT4[:, 256 * m4:256 * (m4 + 1)], kp[:])

            def qproj4(j):
                # q chunk j computed 4x via col groups -> psum already
                # band-replicated; one DVE copy, no DMA.
                qp = pp.tile([P, PC], F32, tag="pp", name="qp")
                nc.vector.memset(qp[:], 0.0)
                nmm = 12 if with_bias else 8
                i = 0
                for h in range(2):
                    for r in range(4):
                        nc.tensor.matmul(
                            qp[32 * r:32 * (r + 1), :], wq_sb[h],
                            xkv_t[h][:, PC * j:PC * (j + 1)],
                            start=False, stop=(i == nmm - 1),
                            skip_group_check=True, tile_position=(0, 32 * r))
                        i += 1
                if with_bias:
                    for r in range(4):
                        nc.tensor.matmul(
                            qp[32 * r:32 * (r + 1), :], bq_sb, ones_sb,
                            start=False, stop=(i == nmm - 1),
                            skip_group_check=True, tile_position=(0, 32 * r))
                        i += 1
                nc.vector.tensor_copy(qrep[:, PC * j:PC * (j + 1)], qp[:])

            def qsolo(j):
                # q chunk j (throughput path): 2 matmuls into band 0, then
                # SWDGE replication to bands 1-3 (latency-tolerant).
                qp = pp.tile([D, PC], F32, tag="pp", name="qs")
                nc.tensor.matmul(qp[:], wq_sb[0], xkv_t[0][:, PC * j:PC * (j + 1)],
                                 start=True, stop=False)
                nc.tensor.matmul(qp[:], wq_sb[1], xkv_t[1][:, PC * j:PC * (j + 1)],
                                 start=False, stop=not with_bias)
                if with_bias:
                    nc.tensor.matmul(qp[:], bq_sb, ones_sb, start=False, stop=True)
                nc.vector.tensor_copy(qrep[0:D, PC * j:PC * (j + 1)], qp[:])
                for r in range(1, 4):
                    nc.gpsimd.dma_start(qrep[32 * r:32 * r + 32, PC * j:PC * (j + 1)],
                                        qrep[0:D, PC * j:PC * (j + 1)])

            def vproj(t):
                j, off = divmod(t, PC // P)
                vp = pp.tile([P, C], F32, tag="pp", name="vp")
                for xh in range(2):
                    nc.tensor.matmul(
                        vp[:], xkv_t[xh][:, PC * j + off * P:PC * j + (off + 1) * P],
                        wv_sb[xh], start=(xh == 0), stop=(xh == 1))
                nc.vector.tensor_copy(v_sb[t][:, 0:C], vp[:])

            # v tiles in AV consumption order: step group g=(m4,t) uses
            # key tile kt = 2*(4*(g//2)+r) + g%2 on band r.
            def kt_of(g, r):
                return 2 * (4 * (g // 2) + r) + (g % 2)
            vorder = [kt_of(g, r) for g in range(8) for r in range(4)]
            V = [lambda t=t: vproj(t) for t in vorder]
            proj_work = ([lambda: kproj256(1), lambda: qsolo(1)] + V[0:3]
                         + V[3:4] + [lambda: qsolo(2)] + V[4:7]
                         + [lambda: kproj256(2), lambda: qsolo(3)] + V[7:10]
                         + V[10:15]
                         + [lambda: kproj256(3)] + V[15:19]
                         + V[19:32])

            # ---- attention: 32 QK steps, AV lags by 1 ----
            def av_norm(op, ci, qb, last=False, act=False):
                rinv = ep.tile([P, 1], F32, tag="rinv", name="rinv")
                nc.vector.reciprocal(rinv[:], op[:, C:C + 1])
                osb = ep.tile([P, C], F16, tag="osb", name="osb")
                if act:
                    # ScalarE is idle after the final exp; runs in parallel
                    # with the DVE half of the last chunk's epilogue.
                    nc.scalar.mul(osb[:], op[:, 0:C], rinv[:])
                else:
                    nc.vector.tensor_scalar_mul(osb[:], op[:, 0:C], rinv[:])
                q0 = ci * 512 + qb * P
                e = nc.scalar if last else nc.sync
                e.dma_start(o_d[q0:q0 + P, :], osb[:])

            pair_tiles = {}
            opsA = {}
            opsB = {}
            wi = 0
            NST32 = 32
            opsBd = {}

            def qk_quad(s):
                ci, g = divmod(s, 8)
                m4g, t = divmod(g, 2)
                sc = ps.tile([P, 2048], F32, tag="p", name="sc")
                for r in range(4):
                    nc.tensor.matmul(
                        sc[:, 512 * r:512 * (r + 1)],
                        kT4[32 * r:32 * r + 32, 256 * m4g + P * t:256 * m4g + P * (t + 1)],
                        qrep[32 * r:32 * r + 32, PC * ci:PC * (ci + 1)],
                        start=True, stop=True, tile_position=(32 * r, 0))
                return sc

            def av_half(s2, rlist, nq):
                ci2, g2 = divmod(s2, 8)
                Pt2 = pair_tiles[s2]
                for r in rlist:
                    kt = kt_of(g2, r)
                    for qs in range(nq):
                        op = opsA[ci2][qs] if qs < 2 else opsB[3][qs - 2]
                        nc.tensor.matmul(
                            op[:],
                            Pt2[:, 512 * r + P * qs:512 * r + P * (qs + 1)],
                            v_sb[kt][:],
                            start=(g2 == 0 and r == 0),
                            stop=(g2 == 7 and r == 3))

            # ---- prologue: QK(0) needs only k block 0 (x cols 0:1024)
            # and q chunk 0; the uniform loop starts at s=0 ----
            qproj4(0)
            kproj256(0)

            for s in range(NST32 + 1):
                qk = s < NST32
                if qk:
                    # all 4 QK matmuls adjacent: K=32 row-banded matmuls
                    # co-execute in disjoint PE quadrants only when nothing
                    # full-array sits between them.
                    sc = qk_quad(s)
                    Pt = big.tile([P, 2048], BF16, tag="big", name="pt")
                    nc.scalar.activation(Pt[:], sc[:], EXP)
                    pair_tiles[s] = Pt
                for _ in range(5):
                    if wi < len(proj_work):
                        proj_work[wi]()
                        wi += 1
                if s >= 1:
                    s2 = s - 1
                    ci2, g2 = divmod(s2, 8)
                    if g2 == 0:
                        opsA[ci2] = [pav.tile([P, VW], F32, tag="av", name="avo")
                                     for _ in range(2)]
                        if ci2 == 3:
                            opsB[3] = [pp.tile([P, VW], F32, tag="pp", name="avb")
                                       for _ in range(2)]
                    nq = 4 if ci2 == 3 else 2
                    av_half(s2, [0, 1], nq)
                    av_half(s2, [2, 3], nq)
                    if ci2 == 3:
                        pair_tiles.pop(s2)
                    if g2 == 7:
                        for qs in range(2):
                            av_norm(opsA[ci2][qs], ci2, qs,
                                    act=(ci2 == 3 and qs == 1))
                        del opsA[ci2]
                        if ci2 == 3:
                            av_norm(opsB[3][0], 3, 2)
                            av_norm(opsB[3][1], 3, 3, last=True, act=True)
                            del opsB[3]
                # B-sweeps for chunks 0..2 (after their A sweep ends):
                # chunks 0/1 are emitted in two 4-key-group halves (at the
                # window start and mid) so the second half lands in the
                # otherwise chain-gapped later steps; chunk 2 stays one
                # burst (its pp tiles must be normed before opsB[3]
                # allocates at s=25).
                for c3, s0 in ((0, 9), (1, 17), (2, 24)):
                    half = 8 if c3 == 2 else 2
                    if s == s0 or (c3 != 2 and s in (s0 + 2, s0 + 4, s0 + 6)):
                        g2lo = 0 if s == s0 else 2 * ((s - s0) // 2)
                        if g2lo == 0:
                            opsBd[c3] = [pp.tile([P, VW], F32, tag="pp",
                                                 name="avb")
                                         for _ in range(2)]
                        for g2 in range(g2lo, g2lo + half):
                            Pt2 = pair_tiles.pop(8 * c3 + g2)
                            for r in range(4):
                                kt = kt_of(g2, r)
                                for qs in range(2):
                                    nc.tensor.matmul(
                                        opsBd[c3][qs][:],
                                        Pt2[:, 512 * r + P * (qs + 2):512 * r + P * (qs + 3)],
                                        v_sb[kt][:],
                                        start=(g2 == 0 and r == 0),
                                        stop=(g2 == 7 and r == 3))
                        if g2lo + half == 8:
                            for qs in range(2):
                                av_norm(opsBd[c3][qs], c3, qs + 2)
                            del opsBd[c3]

    nc.compile()
    return nc


def _in_maps(x, Wq, bq, Wk, bk, Wv, bv):
    xf = np.ascontiguousarray(np.asarray(x, np.float32).reshape(B, C, HW)).astype(np.float16)
    wpk = np.concatenate([
        np.asarray(Wq, np.float32).T,
        np.asarray(Wk, np.float32).T,
        np.asarray(Wv, np.float32).T], axis=1).astype(np.float16)
    bpk = np.concatenate([
        np.asarray(bq, np.float32).reshape(1, D),
        np.asarray(bk, np.float32).reshape(1, D),
        np.ones((1, PC), np.float32)], axis=1).astype(np.float16)
    maps = []
    for core in range(NCORES):
        b, h = divmod(core, 2)
        xroll = np.concatenate([xf[b][:, h * Q:], xf[b][:, :h * Q]], axis=1)
        maps.append({
            "xkv": np.ascontiguousarray(xroll),
            "wpk": np.ascontiguousarray(wpk),
            "bpk": np.ascontiguousarray(bpk),
        })
    return maps


def _gather(results, bv):
    out = np.empty((B, C, HW), np.float32)
    for core in range(NCORES):
        b, h = divmod(core, 2)
        out[b][:, h * Q:(h + 1) * Q] = results[core]["o"].T
    out += np.asarray(bv, np.float32).reshape(1, C, 1)
    return out.reshape(B, C, H, W)


def run(x, Wq, bq, Wk, bk, Wv, bv, **kwargs):
    with_bias = bool(np.any(np.asarray(bq)) or np.any(np.asarray(bk)))
    key = f"nc{int(with_bias)}"
    nc = _CACHE.get(key)
    if nc is None:
        nc = build_program(with_bias=with_bias)
        _CACHE[key] = nc
    maps = _in_maps(x, Wq, bq, Wk, bk, Wv, bv)
    import concourse.mybir as _mb
    wanted = set()
    for a in nc.m.functions[0].allocations:
        if isinstance(a, _mb.MemoryLocationSet) and a.kind == "ExternalInput":
            wanted.add(a.memorylocations[0].name)
    maps = [{k: v for k, v in m.items() if k in wanted} for m in maps]
    res = run_bass_kernel_spmd(nc, maps, core_ids=list(range(NCORES)), **kwargs)
    return _gather(res.results, bv), res


def kernel(x, Wq, bq, Wk, bk, Wv, bv) -> np.ndarray:
    out, _ = run(x, Wq, bq, Wk, bk, Wv, bv)
    return out
